# revision 17
# baseline (speedup 1.0000x reference)
"""MoE layer (8 experts, top-2) for 8 Trainium2 NeuronCores.

Strategy: expert-parallel. The router (0.1% of FLOPs) runs on host and
decides the sharding: tokens are gathered by routed expert host-side (the
all-to-all), each core runs one expert's dense MLP
  y = scale * (gelu(x @ W1 + b1) @ W2 + b2)
over the tokens routed to it, and the host scatter-adds the per-expert
partial outputs back.

Kernel structure (per core): all tensors bf16 on the wire (PSUM accumulates
f32), weight-stationary passes over the full token range. Tokens are split
into NB <= 8 blocks of <= 512 (one PSUM bank each). Stage B runs 16
f-chunk passes (8 k-steps x NB blocks, one LDWEIGHTS-worth of weights per
k-step held across all blocks); stage C runs 8 output-chunk passes (16
k-steps x NB blocks). Weight/x DMAs are ordered so the first matmul only
waits on ~0.8MB.
"""

import os

import numpy as np

HIDDEN = 1024
FF = 2 * HIDDEN
NUM_EXPERTS = 8
TOP_K = 2
NCORES = 8
HC = HIDDEN // 128   # 8 k-chunks for stage B / out-chunks for stage C
FC = FF // 128       # 16 f-chunks

# Set by kernel() when MOE_TRACE=1: HW kernel execution time in ns.
LAST_EXEC_NS = None
LAST_RESULTS = None

_PROGRAM_CACHE = {}


def _round_up(v, m):
    return (v + m - 1) // m * m


def _blocks_of(C, blk):
    """Split C tokens into blocks of <=blk; the remainder block goes last
    (bf16 matmuls run full-rate at any moving size, and a small last block
    minimizes the exposed tail epilogue)."""
    blocks = []
    t0 = 0
    while t0 < C:
        b = min(blk, C - t0)
        blocks.append((t0, b))
        t0 += b
    return blocks


def _build_program(C, blk):
    import concourse.bass as bass  # noqa: F401
    import concourse.mybir as mybir
    import concourse.tile as tile
    from concourse import bacc

    f32 = mybir.dt.float32
    bf16 = mybir.dt.bfloat16

    nc = bacc.Bacc("TRN2", target_bir_lowering=False, debug=False,
                   num_devices=NCORES)

    # DRAM layouts (host-packed, partition-major so every DMA reads >=2KB
    # contiguous per partition):
    #   xp [128, HC, C]       bf16: xp[p, hc, t] = x_t[hc*128+p]
    #   w1p[128, FC, HC, 128] bf16: w1p[p, fc, hc, j] = W1[hc*128+p, fc*128+j]
    #   w2p[128, OC, FC, 128] bf16: w2p[p, oc, fc, j] = W2[fc*128+p, oc*128+j]
    #   yT [128, OC, C]       f32:  y_t[oc*128+p] = yT[p, oc, t]
    xp = nc.dram_tensor("xp", [128, HC, C], bf16, kind="ExternalInput")
    w1 = nc.dram_tensor("w1p", [128, FC, HC, 128], bf16, kind="ExternalInput")
    b1 = nc.dram_tensor("b1", [FF], f32, kind="ExternalInput")
    w2 = nc.dram_tensor("w2p", [128, HC, FC, 128], bf16, kind="ExternalInput")
    scl = nc.dram_tensor("scl", [C], f32, kind="ExternalInput")
    yT = nc.dram_tensor("yT", [128, HC, C], bf16, kind="ExternalOutput")

    blocks = _blocks_of(C, blk)
    NB = len(blocks)
    assert NB <= 8, f"need one PSUM bank per block, got {NB}"
    # Column groups for stage B: passes over group 0 (one block) start as
    # soon as its x columns land; its 16 short passes hide the w1 stream,
    # whose arrival rate then outpaces one pass per fc.
    groups = [blocks[:1], blocks[1:]] if NB > 1 else [blocks]
    spans = [(g[0][0], g[-1][0] + g[-1][1]) for g in groups]

    Gelu = mybir.ActivationFunctionType.Gelu
    Ident = mybir.ActivationFunctionType.Identity

    with tile.TileContext(nc) as tc:
        with (
            tc.tile_pool(name="wts", bufs=1) as wts,
            tc.tile_pool(name="xin", bufs=1) as xin,
            tc.tile_pool(name="hmid", bufs=1) as hmid,
            tc.tile_pool(name="outs", bufs=4) as outs,
            tc.tile_pool(name="ps", bufs=8, space="PSUM") as ps,
        ):
            # --- DMA issue order on the sync ring: first matmul's deps
            # first (w1[fc=0], x group 0), then w1/x interleaved in
            # consumption order, then w2 (needed ~halfway).
            w1_sb = [None] * FC
            x_sb = [[None] * len(groups) for _ in range(HC)]

            # PE warm-up: ~12 matmuls on scratch data issued while the
            # first real operands stream in, so DVFS has ramped the PE to
            # full clock by the time real work starts.
            warm = xin.tile([128, 512], bf16, tag="warm", name="warm")
            nc.gpsimd.memset(warm[:], 0)
            for wi in range(12):
                pw = ps.tile([128, blk], f32, tag="ps", name=f"warmp{wi}")
                nc.tensor.matmul(pw[:, :512], warm[:, :128], warm[:])

            def load_w1(fc):
                t = wts.tile([128, HC, 128], bf16, tag=f"w1f{fc}", name=f"w1f{fc}")
                nc.sync.dma_start(out=t[:], in_=w1.ap()[:, fc])
                w1_sb[fc] = t

            def load_x(hc, gi):
                c0, c1 = spans[gi]
                t = xin.tile([128, c1 - c0], bf16, tag=f"x{hc}_{gi}",
                             name=f"x{hc}_{gi}")
                nc.sync.dma_start(out=t[:], in_=xp.ap()[:, hc, c0:c1])
                x_sb[hc][gi] = t

            def xs(hc, t0, bs):
                # slice of x for block starting at t0 (inside one group)
                for gi, (c0, c1) in enumerate(spans):
                    if c0 <= t0 < c1:
                        return x_sb[hc][gi][:, t0 - c0:t0 - c0 + bs]
                raise AssertionError

            # fc=0 weights split so the very first matmul only waits on a
            # 32KB w1 slice + one 128-col x slice.
            w1f0a = wts.tile([128, 1, 128], bf16, tag="w1f0a", name="w1f0a")
            nc.sync.dma_start(out=w1f0a[:], in_=w1.ap()[:, 0, 0:1])
            w1f0b = wts.tile([128, HC - 1, 128], bf16, tag="w1f0b",
                             name="w1f0b")
            nc.sync.dma_start(out=w1f0b[:], in_=w1.ap()[:, 0, 1:])
            for hc in range(HC):
                load_x(hc, 0)
            for fc in range(1, FC):
                load_w1(fc)
            if len(groups) > 1:
                for hc in range(HC):
                    load_x(hc, 1)
            w2_sb = []
            for oc in range(HC):
                t = wts.tile([128, FC, 128], bf16, tag=f"w2o{oc}", name=f"w2o{oc}")
                nc.sync.dma_start(out=t[:], in_=w2.ap()[:, oc])
                w2_sb.append(t)

            # small stuff on the scalar ring; scl comes in as one 8.6KB
            # row and is broadcast on-chip (a 128-way broadcast DMA was a
            # 1.1MB packet storm competing with the critical x stream).
            # b2 is applied host-side (rank-1 update s (x) b2), so stage C
            # needs no bias and its epilogue is one vector op from PSUM.
            b1_sb = wts.tile([128, FC], f32, tag="b1")
            nc.scalar.dma_start(
                out=b1_sb[:], in_=b1.ap().rearrange("(c p) -> p c", p=128))
            s1_sb = wts.tile([1, C], f32, tag="scl1")
            nc.scalar.dma_start(
                out=s1_sb[:], in_=scl.ap().partition_broadcast(1))
            s_sb = wts.tile([128, C], f32, tag="scl")
            nc.gpsimd.partition_broadcast(out_ap=s_sb[:], in_ap=s1_sb[:])

            h_sb = [hmid.tile([128, C], bf16, tag=f"h{fc}", name=f"h{fc}")
                    for fc in range(FC)]

            # --- Stage B: h[fc] = gelu(sum_hc w1[hc,fc].T @ x[hc] + b1[fc])
            for gi, gblocks in enumerate(groups):
                for fc in range(FC):
                    phs = [ps.tile([128, blk], f32, tag="ps",
                                   name=f"psB{gi}_{fc}_{b}")
                           for b in range(len(gblocks))]
                    for hc in range(HC):
                        if fc == 0:
                            lhsT = w1f0a[:, 0] if hc == 0 else w1f0b[:, hc - 1]
                        else:
                            lhsT = w1_sb[fc][:, hc]
                        for b, (t0, bs) in enumerate(gblocks):
                            nc.tensor.matmul(
                                phs[b][:, :bs],
                                lhsT,
                                xs(hc, t0, bs),
                                start=(hc == 0), stop=(hc == HC - 1),
                            )
                    for b, (t0, bs) in enumerate(gblocks):
                        nc.scalar.activation(
                            out=h_sb[fc][:, t0:t0 + bs], in_=phs[b][:, :bs],
                            func=Gelu, bias=b1_sb[:, fc:fc + 1], scale=1.0)

            # --- Stage C: y[oc] = scl * (sum_fc w2[fc,oc].T @ h[fc] + b2[oc])
            # Block-outer so each block's epilogue (act -> mul -> DMA out,
            # on scalar/vector/gpsimd) pipelines under the next block's
            # matmul stream; only the last block's epilogue is exposed.
            for oc in range(HC):
                for b, (t0, bs) in enumerate(blocks):
                    py = ps.tile([128, blk], f32, tag="ps", name=f"psC{oc}_{b}")
                    for fc in range(FC):
                        nc.tensor.matmul(
                            py[:, :bs],
                            w2_sb[oc][:, fc],
                            h_sb[fc][:, t0:t0 + bs],
                            start=(fc == 0), stop=(fc == FC - 1),
                        )
                    o1 = outs.tile([128, blk], bf16, tag="o1", name=f"o{oc}_{b}")
                    nc.vector.tensor_mul(
                        o1[:, :bs], py[:, :bs], s_sb[:, t0:t0 + bs])
                    nc.scalar.dma_start(
                        out=yT.ap()[:, oc, t0:t0 + bs], in_=o1[:, :bs])

    nc.compile()
    return nc


def _route_host(x, Wr, br):
    """Replicate the reference router bit-exactly (jax on CPU), with a
    numpy fallback (same math, same tie semantics) if jax-cpu is absent."""
    try:
        import jax
        import jax.numpy as jnp

        cpu = jax.devices("cpu")[0]
        xj = jax.device_put(x, cpu)
        Wrj = jax.device_put(Wr, cpu)
        brj = jax.device_put(br, cpu)
        with jax.default_device(cpu):
            logits = jnp.einsum("bsh,he->bse", xj, Wrj) + brj
            routing = jax.nn.softmax(logits, axis=-1)
            topw, topi = jax.lax.top_k(routing, TOP_K)
            topw = jax.nn.softmax(topw, axis=-1)
        return np.asarray(topw), np.asarray(topi)
    except Exception:
        lg = x.reshape(-1, x.shape[-1]).astype(np.float32) @ Wr + br
        m = lg.max(axis=-1, keepdims=True)
        p = np.exp(lg - m)
        p /= p.sum(axis=-1, keepdims=True)
        # top-k with lower-index-wins tie semantics (jax.lax.top_k)
        topi = np.argsort(-p, axis=-1, kind="stable")[:, :TOP_K]
        topv = np.take_along_axis(p, topi, axis=-1)
        e = np.exp(topv - topv.max(axis=-1, keepdims=True))
        topw = (e / e.sum(axis=-1, keepdims=True)).astype(np.float32)
        B, S = x.shape[0], x.shape[1]
        return (topw.reshape(B, S, TOP_K),
                topi.astype(np.int32).reshape(B, S, TOP_K))


def kernel(x, Wr, br, W1, b1, W2, b2):
    global LAST_EXEC_NS, LAST_RESULTS
    import ml_dtypes
    from concourse.bass_utils import run_bass_kernel_spmd

    bf16 = ml_dtypes.bfloat16

    x = np.ascontiguousarray(np.asarray(x, dtype=np.float32))
    Wr = np.asarray(Wr, dtype=np.float32)
    br = np.asarray(br, dtype=np.float32)
    W1 = np.ascontiguousarray(np.asarray(W1, dtype=np.float32))
    b1 = np.ascontiguousarray(np.asarray(b1, dtype=np.float32))
    W2 = np.ascontiguousarray(np.asarray(W2, dtype=np.float32))
    b2 = np.ascontiguousarray(np.asarray(b2, dtype=np.float32))

    B, S, H = x.shape
    ntok = B * S
    xf = x.reshape(ntok, H)

    topw, topi = _route_host(x, Wr, br)
    topw = topw.reshape(ntok, TOP_K)
    topi = topi.reshape(ntok, TOP_K)

    # per-expert token index lists + combine weights
    idx = []
    wgt = []
    for e in range(NUM_EXPERTS):
        mask = (topi == e)
        tok = np.nonzero(mask.any(axis=1))[0]
        w = (topw * mask).sum(axis=1)[tok].astype(np.float32)
        idx.append(tok)
        wgt.append(w)
    counts = np.array([len(t) for t in idx])

    blk = int(os.environ.get("MOE_BLK", "512"))
    C = max(_round_up(int(counts.max()), 2), 512)

    key = (C, blk)
    if key not in _PROGRAM_CACHE:
        _PROGRAM_CACHE[key] = _build_program(C, blk)
    nc = _PROGRAM_CACHE[key]

    in_maps = []
    for e in range(NUM_EXPERTS):
        cnt = counts[e]
        xpe = np.zeros((C, H), dtype=np.float32)
        xpe[:cnt] = xf[idx[e]]
        # xp[p, hc, t] = x_t[hc*128+p]
        xpe = np.ascontiguousarray(
            xpe.T.reshape(HC, 128, C).transpose(1, 0, 2).astype(bf16))
        scle = np.zeros((C,), dtype=np.float32)
        scle[:cnt] = wgt[e]
        # w1p[p, fc, hc, j] = W1[e][hc*128+p, fc*128+j]
        w1p = np.ascontiguousarray(
            W1[e].reshape(HC, 128, FC, 128).transpose(1, 2, 0, 3).astype(bf16))
        # w2p[p, oc, fc, j] = W2[e][fc*128+p, oc*128+j]
        w2p = np.ascontiguousarray(
            W2[e].reshape(FC, 128, HC, 128).transpose(1, 2, 0, 3).astype(bf16))
        in_maps.append({
            "xp": xpe,
            "w1p": w1p,
            "b1": np.ascontiguousarray(b1[e]),
            "w2p": w2p,
            "scl": scle,
        })

    trace = os.environ.get("MOE_TRACE", "0") == "1"
    res = run_bass_kernel_spmd(
        nc, in_maps, core_ids=list(range(NCORES)), trace=trace)
    LAST_EXEC_NS = res.exec_time_ns
    LAST_RESULTS = res

    out = np.zeros((ntok, H), dtype=np.float32)
    for e in range(NUM_EXPERTS):
        cnt = counts[e]
        ye = np.asarray(res.results[e]["yT"], dtype=np.float32)  # [128, HC, C]
        ye = ye.transpose(1, 0, 2).reshape(H, C)[:, :cnt].T  # [cnt, H]
        # b2 is folded in host-side: y += s * b2 (rank-1 update)
        out[idx[e]] += ye + wgt[e][:, None] * b2[e][None, :]
    return out.reshape(B, S, H)


# revision 20
# speedup vs baseline: 1.0016x; 1.0016x over previous
"""MoE layer (8 experts, top-2) for 8 Trainium2 NeuronCores.

Strategy: expert-parallel. The router (0.1% of FLOPs) runs on host and
decides the sharding: tokens are gathered by routed expert host-side (the
all-to-all), each core runs one expert's dense MLP
  y = scale * (gelu(x @ W1 + b1) @ W2 + b2)
over the tokens routed to it, and the host scatter-adds the per-expert
partial outputs back.

Kernel structure (per core): all tensors bf16 on the wire (PSUM accumulates
f32), weight-stationary passes over the full token range. Tokens are split
into NB <= 8 blocks of <= 512 (one PSUM bank each). Stage B runs 16
f-chunk passes (8 k-steps x NB blocks, one LDWEIGHTS-worth of weights per
k-step held across all blocks); stage C runs 8 output-chunk passes (16
k-steps x NB blocks). Weight/x DMAs are ordered so the first matmul only
waits on ~0.8MB.
"""

import os

import numpy as np

HIDDEN = 1024
FF = 2 * HIDDEN
NUM_EXPERTS = 8
TOP_K = 2
NCORES = 8
HC = HIDDEN // 128   # 8 k-chunks for stage B / out-chunks for stage C
FC = FF // 128       # 16 f-chunks

# Set by kernel() when MOE_TRACE=1: HW kernel execution time in ns.
LAST_EXEC_NS = None
LAST_RESULTS = None

_PROGRAM_CACHE = {}


def _round_up(v, m):
    return (v + m - 1) // m * m


def _blocks_of(C, blk):
    """Split C tokens into blocks of <=blk; the remainder block goes last
    (bf16 matmuls run full-rate at any moving size, and a small last block
    minimizes the exposed tail epilogue)."""
    blocks = []
    t0 = 0
    while t0 < C:
        b = min(blk, C - t0)
        blocks.append((t0, b))
        t0 += b
    return blocks


def _build_program(C, blk):
    import concourse.bass as bass  # noqa: F401
    import concourse.mybir as mybir
    import concourse.tile as tile
    from concourse import bacc

    f32 = mybir.dt.float32
    bf16 = mybir.dt.bfloat16

    nc = bacc.Bacc("TRN2", target_bir_lowering=False, debug=False,
                   num_devices=NCORES)

    # DRAM layouts (host-packed, partition-major so every DMA reads >=2KB
    # contiguous per partition):
    #   xp [128, HC, C]       bf16: xp[p, hc, t] = x_t[hc*128+p]
    #   w1p[128, FC, HC, 128] bf16: w1p[p, fc, hc, j] = W1[hc*128+p, fc*128+j]
    #   w2p[128, OC, FC, 128] bf16: w2p[p, oc, fc, j] = W2[fc*128+p, oc*128+j]
    #   yT [128, OC, C]       f32:  y_t[oc*128+p] = yT[p, oc, t]
    xp = nc.dram_tensor("xp", [128, HC, C], bf16, kind="ExternalInput")
    w1 = nc.dram_tensor("w1p", [128, FC, HC, 128], bf16, kind="ExternalInput")
    b1 = nc.dram_tensor("b1", [FF], f32, kind="ExternalInput")
    w2 = nc.dram_tensor("w2p", [128, HC, FC, 128], bf16, kind="ExternalInput")
    scl = nc.dram_tensor("scl", [C], f32, kind="ExternalInput")
    yT = nc.dram_tensor("yT", [128, HC, C], bf16, kind="ExternalOutput")

    blocks = _blocks_of(C, blk)
    NB = len(blocks)
    assert NB <= 8, f"need one PSUM bank per block, got {NB}"
    # Column groups for stage B: passes over group 0 (one block) start as
    # soon as its x columns land; its 16 short passes hide the w1 stream,
    # whose arrival rate then outpaces one pass per fc.
    groups = [blocks[:1], blocks[1:]] if NB > 1 else [blocks]
    spans = [(g[0][0], g[-1][0] + g[-1][1]) for g in groups]

    Gelu = mybir.ActivationFunctionType.Gelu
    Ident = mybir.ActivationFunctionType.Identity

    with tile.TileContext(nc) as tc:
        with (
            tc.tile_pool(name="wts", bufs=1) as wts,
            tc.tile_pool(name="xin", bufs=1) as xin,
            tc.tile_pool(name="hmid", bufs=1) as hmid,
            tc.tile_pool(name="outs", bufs=4) as outs,
            tc.tile_pool(name="ps", bufs=8, space="PSUM") as ps,
        ):
            # --- DMA issue order on the sync ring: first matmul's deps
            # first (w1[fc=0], x group 0), then w1/x interleaved in
            # consumption order, then w2 (needed ~halfway).
            w1_sb = [None] * FC

            # PE warm-up: ~12 matmuls on scratch data issued while the
            # first real operands stream in, so DVFS has ramped the PE to
            # full clock by the time real work starts.
            warm = xin.tile([128, 512], bf16, tag="warm", name="warm")
            nc.gpsimd.memset(warm[:], 0)
            for wi in range(7):
                pw = ps.tile([128, blk], f32, tag="ps", name=f"warmp{wi}")
                nc.tensor.matmul(pw[:, :512], warm[:, :128], warm[:])

            def load_w1(fc):
                t = wts.tile([128, HC, 128], bf16, tag=f"w1f{fc}", name=f"w1f{fc}")
                nc.sync.dma_start(out=t[:], in_=w1.ap()[:, fc])
                w1_sb[fc] = t

            # x arrives in few, large DMAs: descriptor issue costs ~600ns
            # on the ring engine, so 8 per-hc transfers were issue-bound.
            xg_sb = [None] * len(groups)

            def load_xg(gi, nsplit):
                c0, c1 = spans[gi]
                t = xin.tile([128, HC, c1 - c0], bf16, tag=f"xg{gi}",
                             name=f"xg{gi}")
                step = HC // nsplit
                for s in range(nsplit):
                    nc.sync.dma_start(
                        out=t[:, s * step:(s + 1) * step],
                        in_=xp.ap()[:, s * step:(s + 1) * step, c0:c1])
                xg_sb[gi] = t

            def xs(hc, t0, bs):
                # slice of x for block starting at t0 (inside one group)
                for gi, (c0, c1) in enumerate(spans):
                    if c0 <= t0 < c1:
                        return xg_sb[gi][:, hc, t0 - c0:t0 - c0 + bs]
                raise AssertionError

            # fc=0 weights split so the very first matmul only waits on a
            # 32KB w1 slice + the group-0 x tile.
            w1f0a = wts.tile([128, 1, 128], bf16, tag="w1f0a", name="w1f0a")
            nc.sync.dma_start(out=w1f0a[:], in_=w1.ap()[:, 0, 0:1])
            w1f0b = wts.tile([128, HC - 1, 128], bf16, tag="w1f0b",
                             name="w1f0b")
            nc.sync.dma_start(out=w1f0b[:], in_=w1.ap()[:, 0, 1:])
            load_xg(0, 1)
            for fc in range(1, FC):
                load_w1(fc)
            if len(groups) > 1:
                load_xg(1, 2)
            w2_sb = []
            for oc in range(HC):
                t = wts.tile([128, FC, 128], bf16, tag=f"w2o{oc}", name=f"w2o{oc}")
                nc.sync.dma_start(out=t[:], in_=w2.ap()[:, oc])
                w2_sb.append(t)

            # small stuff on the scalar ring; scl comes in as one 8.6KB
            # row and is broadcast on-chip (a 128-way broadcast DMA was a
            # 1.1MB packet storm competing with the critical x stream).
            # b2 is applied host-side (rank-1 update s (x) b2), so stage C
            # needs no bias and its epilogue is one vector op from PSUM.
            b1_sb = wts.tile([128, FC], f32, tag="b1")
            nc.scalar.dma_start(
                out=b1_sb[:], in_=b1.ap().rearrange("(c p) -> p c", p=128))
            s1_sb = wts.tile([1, C], f32, tag="scl1")
            nc.scalar.dma_start(
                out=s1_sb[:], in_=scl.ap().partition_broadcast(1))
            s_sb = wts.tile([128, C], f32, tag="scl")
            nc.gpsimd.partition_broadcast(out_ap=s_sb[:], in_ap=s1_sb[:])

            h_sb = [hmid.tile([128, C], bf16, tag=f"h{fc}", name=f"h{fc}")
                    for fc in range(FC)]

            # --- Stage B: h[fc] = gelu(sum_hc w1[hc,fc].T @ x[hc] + b1[fc])
            for gi, gblocks in enumerate(groups):
                for fc in range(FC):
                    phs = [ps.tile([128, blk], f32, tag="ps",
                                   name=f"psB{gi}_{fc}_{b}")
                           for b in range(len(gblocks))]
                    for hc in range(HC):
                        if fc == 0:
                            lhsT = w1f0a[:, 0] if hc == 0 else w1f0b[:, hc - 1]
                        else:
                            lhsT = w1_sb[fc][:, hc]
                        for b, (t0, bs) in enumerate(gblocks):
                            nc.tensor.matmul(
                                phs[b][:, :bs],
                                lhsT,
                                xs(hc, t0, bs),
                                start=(hc == 0), stop=(hc == HC - 1),
                            )
                    for b, (t0, bs) in enumerate(gblocks):
                        nc.scalar.activation(
                            out=h_sb[fc][:, t0:t0 + bs], in_=phs[b][:, :bs],
                            func=Gelu, bias=b1_sb[:, fc:fc + 1], scale=1.0)

            # --- Stage C: y[oc] = scl * (sum_fc w2[fc,oc].T @ h[fc] + b2[oc])
            # Block-outer so each block's epilogue (act -> mul -> DMA out,
            # on scalar/vector/gpsimd) pipelines under the next block's
            # matmul stream; only the last block's epilogue is exposed.
            for oc in range(HC):
                for b, (t0, bs) in enumerate(blocks):
                    py = ps.tile([128, blk], f32, tag="ps", name=f"psC{oc}_{b}")
                    for fc in range(FC):
                        nc.tensor.matmul(
                            py[:, :bs],
                            w2_sb[oc][:, fc],
                            h_sb[fc][:, t0:t0 + bs],
                            start=(fc == 0), stop=(fc == FC - 1),
                        )
                    o1 = outs.tile([128, blk], bf16, tag="o1", name=f"o{oc}_{b}")
                    nc.vector.tensor_mul(
                        o1[:, :bs], py[:, :bs], s_sb[:, t0:t0 + bs])
                    nc.scalar.dma_start(
                        out=yT.ap()[:, oc, t0:t0 + bs], in_=o1[:, :bs])

    nc.compile()
    return nc


def _route_host(x, Wr, br):
    """Replicate the reference router bit-exactly (jax on CPU), with a
    numpy fallback (same math, same tie semantics) if jax-cpu is absent."""
    try:
        import jax
        import jax.numpy as jnp

        cpu = jax.devices("cpu")[0]
        xj = jax.device_put(x, cpu)
        Wrj = jax.device_put(Wr, cpu)
        brj = jax.device_put(br, cpu)
        with jax.default_device(cpu):
            logits = jnp.einsum("bsh,he->bse", xj, Wrj) + brj
            routing = jax.nn.softmax(logits, axis=-1)
            topw, topi = jax.lax.top_k(routing, TOP_K)
            topw = jax.nn.softmax(topw, axis=-1)
        return np.asarray(topw), np.asarray(topi)
    except Exception:
        lg = x.reshape(-1, x.shape[-1]).astype(np.float32) @ Wr + br
        m = lg.max(axis=-1, keepdims=True)
        p = np.exp(lg - m)
        p /= p.sum(axis=-1, keepdims=True)
        # top-k with lower-index-wins tie semantics (jax.lax.top_k)
        topi = np.argsort(-p, axis=-1, kind="stable")[:, :TOP_K]
        topv = np.take_along_axis(p, topi, axis=-1)
        e = np.exp(topv - topv.max(axis=-1, keepdims=True))
        topw = (e / e.sum(axis=-1, keepdims=True)).astype(np.float32)
        B, S = x.shape[0], x.shape[1]
        return (topw.reshape(B, S, TOP_K),
                topi.astype(np.int32).reshape(B, S, TOP_K))


def kernel(x, Wr, br, W1, b1, W2, b2):
    global LAST_EXEC_NS, LAST_RESULTS
    import ml_dtypes
    from concourse.bass_utils import run_bass_kernel_spmd

    bf16 = ml_dtypes.bfloat16

    x = np.ascontiguousarray(np.asarray(x, dtype=np.float32))
    Wr = np.asarray(Wr, dtype=np.float32)
    br = np.asarray(br, dtype=np.float32)
    W1 = np.ascontiguousarray(np.asarray(W1, dtype=np.float32))
    b1 = np.ascontiguousarray(np.asarray(b1, dtype=np.float32))
    W2 = np.ascontiguousarray(np.asarray(W2, dtype=np.float32))
    b2 = np.ascontiguousarray(np.asarray(b2, dtype=np.float32))

    B, S, H = x.shape
    ntok = B * S
    xf = x.reshape(ntok, H)

    topw, topi = _route_host(x, Wr, br)
    topw = topw.reshape(ntok, TOP_K)
    topi = topi.reshape(ntok, TOP_K)

    # per-expert token index lists + combine weights
    idx = []
    wgt = []
    for e in range(NUM_EXPERTS):
        mask = (topi == e)
        tok = np.nonzero(mask.any(axis=1))[0]
        w = (topw * mask).sum(axis=1)[tok].astype(np.float32)
        idx.append(tok)
        wgt.append(w)
    counts = np.array([len(t) for t in idx])

    blk = int(os.environ.get("MOE_BLK", "512"))
    C = max(_round_up(int(counts.max()), 2), 512)

    key = (C, blk)
    if key not in _PROGRAM_CACHE:
        _PROGRAM_CACHE[key] = _build_program(C, blk)
    nc = _PROGRAM_CACHE[key]

    in_maps = []
    for e in range(NUM_EXPERTS):
        cnt = counts[e]
        xpe = np.zeros((C, H), dtype=np.float32)
        xpe[:cnt] = xf[idx[e]]
        # xp[p, hc, t] = x_t[hc*128+p]
        xpe = np.ascontiguousarray(
            xpe.T.reshape(HC, 128, C).transpose(1, 0, 2).astype(bf16))
        scle = np.zeros((C,), dtype=np.float32)
        scle[:cnt] = wgt[e]
        # w1p[p, fc, hc, j] = W1[e][hc*128+p, fc*128+j]
        w1p = np.ascontiguousarray(
            W1[e].reshape(HC, 128, FC, 128).transpose(1, 2, 0, 3).astype(bf16))
        # w2p[p, oc, fc, j] = W2[e][fc*128+p, oc*128+j]
        w2p = np.ascontiguousarray(
            W2[e].reshape(FC, 128, HC, 128).transpose(1, 2, 0, 3).astype(bf16))
        in_maps.append({
            "xp": xpe,
            "w1p": w1p,
            "b1": np.ascontiguousarray(b1[e]),
            "w2p": w2p,
            "scl": scle,
        })

    trace = os.environ.get("MOE_TRACE", "0") == "1"
    res = run_bass_kernel_spmd(
        nc, in_maps, core_ids=list(range(NCORES)), trace=trace)
    LAST_EXEC_NS = res.exec_time_ns
    LAST_RESULTS = res

    out = np.zeros((ntok, H), dtype=np.float32)
    for e in range(NUM_EXPERTS):
        cnt = counts[e]
        ye = np.asarray(res.results[e]["yT"], dtype=np.float32)  # [128, HC, C]
        ye = ye.transpose(1, 0, 2).reshape(H, C)[:, :cnt].T  # [cnt, H]
        # b2 is folded in host-side: y += s * b2 (rank-1 update)
        out[idx[e]] += ye + wgt[e][:, None] * b2[e][None, :]
    return out.reshape(B, S, H)


# revision 23
# speedup vs baseline: 1.0064x; 1.0047x over previous
"""MoE layer (8 experts, top-2) for 8 Trainium2 NeuronCores.

Strategy: expert-parallel. The router (0.1% of FLOPs) runs on host and
decides the sharding: tokens are gathered by routed expert host-side (the
all-to-all), each core runs one expert's dense MLP
  y = scale * (gelu(x @ W1 + b1) @ W2 + b2)
over the tokens routed to it, and the host scatter-adds the per-expert
partial outputs back.

Kernel structure (per core): all tensors bf16 on the wire (PSUM accumulates
f32), weight-stationary passes over the full token range. Tokens are split
into NB <= 8 blocks of <= 512 (one PSUM bank each). Stage B runs 16
f-chunk passes (8 k-steps x NB blocks, one LDWEIGHTS-worth of weights per
k-step held across all blocks); stage C runs 8 output-chunk passes (16
k-steps x NB blocks). Weight/x DMAs are ordered so the first matmul only
waits on ~0.8MB.
"""

import os

import numpy as np

HIDDEN = 1024
FF = 2 * HIDDEN
NUM_EXPERTS = 8
TOP_K = 2
NCORES = 8
HC = HIDDEN // 128   # 8 k-chunks for stage B / out-chunks for stage C
FC = FF // 128       # 16 f-chunks

# Set by kernel() when MOE_TRACE=1: HW kernel execution time in ns.
LAST_EXEC_NS = None
LAST_RESULTS = None

_PROGRAM_CACHE = {}


def _round_up(v, m):
    return (v + m - 1) // m * m


def _blocks_of(C, blk):
    """Split C tokens into blocks of <=blk; the remainder block goes last
    (bf16 matmuls run full-rate at any moving size, and a small last block
    minimizes the exposed tail epilogue)."""
    blocks = []
    t0 = 0
    while t0 < C:
        b = min(blk, C - t0)
        blocks.append((t0, b))
        t0 += b
    return blocks


def _build_program(C, blk):
    import concourse.bass as bass  # noqa: F401
    import concourse.mybir as mybir
    import concourse.tile as tile
    from concourse import bacc

    f32 = mybir.dt.float32
    bf16 = mybir.dt.bfloat16

    nc = bacc.Bacc("TRN2", target_bir_lowering=False, debug=False,
                   num_devices=NCORES)

    # DRAM layouts (host-packed, partition-major so every DMA reads >=2KB
    # contiguous per partition):
    #   xp [128, HC, C]       bf16: xp[p, hc, t] = x_t[hc*128+p]
    #   w1p[128, FC, HC, 128] bf16: w1p[p, fc, hc, j] = W1[hc*128+p, fc*128+j]
    #   w2p[128, OC, FC, 128] bf16: w2p[p, oc, fc, j] = W2[fc*128+p, oc*128+j]
    #   yT [128, OC, C]       f32:  y_t[oc*128+p] = yT[p, oc, t]
    S0 = min(512, C)  # group-0 column count (must match _blocks_of)
    xg0d = nc.dram_tensor("xg0", [128, HC, S0], bf16, kind="ExternalInput")
    xg1d = (nc.dram_tensor("xg1", [128, HC, C - S0], bf16,
                           kind="ExternalInput") if C > S0 else None)
    w1 = nc.dram_tensor("w1p", [128, FC, HC, 128], bf16, kind="ExternalInput")
    b1 = nc.dram_tensor("b1", [FF], f32, kind="ExternalInput")
    w2 = nc.dram_tensor("w2p", [128, HC, FC, 128], bf16, kind="ExternalInput")
    scl = nc.dram_tensor("scl", [C], f32, kind="ExternalInput")
    yT = nc.dram_tensor("yT", [128, HC, C], bf16, kind="ExternalOutput")

    blocks = _blocks_of(C, blk)
    NB = len(blocks)
    assert NB <= 8, f"need one PSUM bank per block, got {NB}"
    # Column groups for stage B: passes over group 0 (one block) start as
    # soon as its x columns land; its 16 short passes hide the w1 stream,
    # whose arrival rate then outpaces one pass per fc.
    groups = [blocks[:1], blocks[1:]] if NB > 1 else [blocks]
    spans = [(g[0][0], g[-1][0] + g[-1][1]) for g in groups]

    Gelu = mybir.ActivationFunctionType.Gelu
    Ident = mybir.ActivationFunctionType.Identity

    with tile.TileContext(nc) as tc:
        with (
            tc.tile_pool(name="wts", bufs=1) as wts,
            tc.tile_pool(name="xin", bufs=1) as xin,
            tc.tile_pool(name="hmid", bufs=1) as hmid,
            tc.tile_pool(name="outs", bufs=4) as outs,
            tc.tile_pool(name="ps", bufs=8, space="PSUM") as ps,
        ):
            # --- DMA issue order on the sync ring: first matmul's deps
            # first (w1[fc=0], x group 0), then w1/x interleaved in
            # consumption order, then w2 (needed ~halfway).
            w1_sb = [None] * FC

            # PE warm-up: ~12 matmuls on scratch data issued while the
            # first real operands stream in, so DVFS has ramped the PE to
            # full clock by the time real work starts.
            warm = xin.tile([128, 512], bf16, tag="warm", name="warm")
            nc.gpsimd.memset(warm[:], 0)
            for wi in range(7):
                pw = ps.tile([128, blk], f32, tag="ps", name=f"warmp{wi}")
                nc.tensor.matmul(pw[:, :512], warm[:, :128], warm[:])

            def load_w1(fc):
                t = wts.tile([128, HC, 128], bf16, tag=f"w1f{fc}", name=f"w1f{fc}")
                nc.sync.dma_start(out=t[:], in_=w1.ap()[:, fc])
                w1_sb[fc] = t

            # x arrives in few, large DMAs: descriptor issue costs ~600ns
            # on the ring engine, so 8 per-hc transfers were issue-bound.
            # Each group's x is packed contiguously in its own DRAM tensor
            # (a column-slice of one big tensor reads 1KB strided chunks);
            # fetched in two hc-halves so early k-steps can start sooner.
            xg_sb = [None] * len(groups)

            def load_xg(gi):
                dram = xg0d if gi == 0 else xg1d
                c0, c1 = spans[gi]
                t = xin.tile([128, HC, c1 - c0], bf16, tag=f"xg{gi}",
                             name=f"xg{gi}")
                for s in range(2):
                    h0, h1 = s * (HC // 2), (s + 1) * (HC // 2)
                    nc.sync.dma_start(
                        out=t[:, h0:h1], in_=dram.ap()[:, h0:h1])
                xg_sb[gi] = t

            def xs(hc, t0, bs):
                # slice of x for block starting at t0 (inside one group)
                for gi, (c0, c1) in enumerate(spans):
                    if c0 <= t0 < c1:
                        return xg_sb[gi][:, hc, t0 - c0:t0 - c0 + bs]
                raise AssertionError

            # fc=0 weights split so the very first matmul only waits on a
            # 32KB w1 slice + the group-0 x tile.
            w1f0a = wts.tile([128, 1, 128], bf16, tag="w1f0a", name="w1f0a")
            nc.sync.dma_start(out=w1f0a[:], in_=w1.ap()[:, 0, 0:1])
            w1f0b = wts.tile([128, HC - 1, 128], bf16, tag="w1f0b",
                             name="w1f0b")
            nc.sync.dma_start(out=w1f0b[:], in_=w1.ap()[:, 0, 1:])
            load_xg(0)
            for fc in range(1, FC):
                load_w1(fc)
            if len(groups) > 1:
                load_xg(1)
            w2_sb = []
            for oc in range(HC):
                t = wts.tile([128, FC, 128], bf16, tag=f"w2o{oc}", name=f"w2o{oc}")
                nc.sync.dma_start(out=t[:], in_=w2.ap()[:, oc])
                w2_sb.append(t)

            # small stuff on the scalar ring; scl comes in as one 8.6KB
            # row and is broadcast on-chip (a 128-way broadcast DMA was a
            # 1.1MB packet storm competing with the critical x stream).
            # b2 is applied host-side (rank-1 update s (x) b2), so stage C
            # needs no bias and its epilogue is one vector op from PSUM.
            b1_sb = wts.tile([128, FC], f32, tag="b1")
            nc.scalar.dma_start(
                out=b1_sb[:], in_=b1.ap().rearrange("(c p) -> p c", p=128))
            s1_sb = wts.tile([1, C], f32, tag="scl1")
            nc.scalar.dma_start(
                out=s1_sb[:], in_=scl.ap().partition_broadcast(1))
            s_sb = wts.tile([128, C], f32, tag="scl")
            nc.gpsimd.partition_broadcast(out_ap=s_sb[:], in_ap=s1_sb[:])

            h_sb = [hmid.tile([128, C], bf16, tag=f"h{fc}", name=f"h{fc}")
                    for fc in range(FC)]

            # --- Stage B: h[fc] = gelu(sum_hc w1[hc,fc].T @ x[hc] + b1[fc])
            for gi, gblocks in enumerate(groups):
                for fc in range(FC):
                    phs = [ps.tile([128, blk], f32, tag="ps",
                                   name=f"psB{gi}_{fc}_{b}")
                           for b in range(len(gblocks))]
                    for hc in range(HC):
                        if fc == 0:
                            lhsT = w1f0a[:, 0] if hc == 0 else w1f0b[:, hc - 1]
                        else:
                            lhsT = w1_sb[fc][:, hc]
                        for b, (t0, bs) in enumerate(gblocks):
                            nc.tensor.matmul(
                                phs[b][:, :bs],
                                lhsT,
                                xs(hc, t0, bs),
                                start=(hc == 0), stop=(hc == HC - 1),
                            )
                    for b, (t0, bs) in enumerate(gblocks):
                        nc.scalar.activation(
                            out=h_sb[fc][:, t0:t0 + bs], in_=phs[b][:, :bs],
                            func=Gelu, bias=b1_sb[:, fc:fc + 1], scale=1.0)

            # --- Stage C: y[oc] = scl * (sum_fc w2[fc,oc].T @ h[fc] + b2[oc])
            # Block-outer so each block's epilogue (act -> mul -> DMA out,
            # on scalar/vector/gpsimd) pipelines under the next block's
            # matmul stream; only the last block's epilogue is exposed.
            for oc in range(HC):
                for b, (t0, bs) in enumerate(blocks):
                    py = ps.tile([128, blk], f32, tag="ps", name=f"psC{oc}_{b}")
                    for fc in range(FC):
                        nc.tensor.matmul(
                            py[:, :bs],
                            w2_sb[oc][:, fc],
                            h_sb[fc][:, t0:t0 + bs],
                            start=(fc == 0), stop=(fc == FC - 1),
                        )
                    o1 = outs.tile([128, blk], bf16, tag="o1", name=f"o{oc}_{b}")
                    nc.vector.tensor_mul(
                        o1[:, :bs], py[:, :bs], s_sb[:, t0:t0 + bs])
                    nc.scalar.dma_start(
                        out=yT.ap()[:, oc, t0:t0 + bs], in_=o1[:, :bs])

    nc.compile()
    return nc


def _route_host(x, Wr, br):
    """Replicate the reference router bit-exactly (jax on CPU), with a
    numpy fallback (same math, same tie semantics) if jax-cpu is absent."""
    try:
        import jax
        import jax.numpy as jnp

        cpu = jax.devices("cpu")[0]
        xj = jax.device_put(x, cpu)
        Wrj = jax.device_put(Wr, cpu)
        brj = jax.device_put(br, cpu)
        with jax.default_device(cpu):
            logits = jnp.einsum("bsh,he->bse", xj, Wrj) + brj
            routing = jax.nn.softmax(logits, axis=-1)
            topw, topi = jax.lax.top_k(routing, TOP_K)
            topw = jax.nn.softmax(topw, axis=-1)
        return np.asarray(topw), np.asarray(topi)
    except Exception:
        lg = x.reshape(-1, x.shape[-1]).astype(np.float32) @ Wr + br
        m = lg.max(axis=-1, keepdims=True)
        p = np.exp(lg - m)
        p /= p.sum(axis=-1, keepdims=True)
        # top-k with lower-index-wins tie semantics (jax.lax.top_k)
        topi = np.argsort(-p, axis=-1, kind="stable")[:, :TOP_K]
        topv = np.take_along_axis(p, topi, axis=-1)
        e = np.exp(topv - topv.max(axis=-1, keepdims=True))
        topw = (e / e.sum(axis=-1, keepdims=True)).astype(np.float32)
        B, S = x.shape[0], x.shape[1]
        return (topw.reshape(B, S, TOP_K),
                topi.astype(np.int32).reshape(B, S, TOP_K))


def kernel(x, Wr, br, W1, b1, W2, b2):
    global LAST_EXEC_NS, LAST_RESULTS
    import ml_dtypes
    from concourse.bass_utils import run_bass_kernel_spmd

    bf16 = ml_dtypes.bfloat16

    x = np.ascontiguousarray(np.asarray(x, dtype=np.float32))
    Wr = np.asarray(Wr, dtype=np.float32)
    br = np.asarray(br, dtype=np.float32)
    W1 = np.ascontiguousarray(np.asarray(W1, dtype=np.float32))
    b1 = np.ascontiguousarray(np.asarray(b1, dtype=np.float32))
    W2 = np.ascontiguousarray(np.asarray(W2, dtype=np.float32))
    b2 = np.ascontiguousarray(np.asarray(b2, dtype=np.float32))

    B, S, H = x.shape
    ntok = B * S
    xf = x.reshape(ntok, H)

    topw, topi = _route_host(x, Wr, br)
    topw = topw.reshape(ntok, TOP_K)
    topi = topi.reshape(ntok, TOP_K)

    # per-expert token index lists + combine weights
    idx = []
    wgt = []
    for e in range(NUM_EXPERTS):
        mask = (topi == e)
        tok = np.nonzero(mask.any(axis=1))[0]
        w = (topw * mask).sum(axis=1)[tok].astype(np.float32)
        idx.append(tok)
        wgt.append(w)
    counts = np.array([len(t) for t in idx])

    blk = int(os.environ.get("MOE_BLK", "512"))
    C = max(_round_up(int(counts.max()), 2), 512)

    key = (C, blk)
    if key not in _PROGRAM_CACHE:
        _PROGRAM_CACHE[key] = _build_program(C, blk)
    nc = _PROGRAM_CACHE[key]

    in_maps = []
    for e in range(NUM_EXPERTS):
        cnt = counts[e]
        xpe = np.zeros((C, H), dtype=np.float32)
        xpe[:cnt] = xf[idx[e]]
        # x[p, hc, t] = x_t[hc*128+p], split into per-group tensors
        xpe = xpe.T.reshape(HC, 128, C).transpose(1, 0, 2).astype(bf16)
        S0 = min(512, C)
        xg0 = np.ascontiguousarray(xpe[:, :, :S0])
        xg1 = np.ascontiguousarray(xpe[:, :, S0:])
        scle = np.zeros((C,), dtype=np.float32)
        scle[:cnt] = wgt[e]
        # w1p[p, fc, hc, j] = W1[e][hc*128+p, fc*128+j]
        w1p = np.ascontiguousarray(
            W1[e].reshape(HC, 128, FC, 128).transpose(1, 2, 0, 3).astype(bf16))
        # w2p[p, oc, fc, j] = W2[e][fc*128+p, oc*128+j]
        w2p = np.ascontiguousarray(
            W2[e].reshape(FC, 128, HC, 128).transpose(1, 2, 0, 3).astype(bf16))
        im = {
            "xg0": xg0,
            "w1p": w1p,
            "b1": np.ascontiguousarray(b1[e]),
            "w2p": w2p,
            "scl": scle,
        }
        if C > S0:
            im["xg1"] = xg1
        in_maps.append(im)

    trace = os.environ.get("MOE_TRACE", "0") == "1"
    res = run_bass_kernel_spmd(
        nc, in_maps, core_ids=list(range(NCORES)), trace=trace)
    LAST_EXEC_NS = res.exec_time_ns
    LAST_RESULTS = res

    out = np.zeros((ntok, H), dtype=np.float32)
    for e in range(NUM_EXPERTS):
        cnt = counts[e]
        ye = np.asarray(res.results[e]["yT"], dtype=np.float32)  # [128, HC, C]
        ye = ye.transpose(1, 0, 2).reshape(H, C)[:, :cnt].T  # [cnt, H]
        # b2 is folded in host-side: y += s * b2 (rank-1 update)
        out[idx[e]] += ye + wgt[e][:, None] * b2[e][None, :]
    return out.reshape(B, S, H)
